# Initial kernel scaffold
#
"""nn_NeuralODE TRN2 kernel: 100 SSP-RK3 steps of a learned 16-channel
1D stencil (conv k=5) with ghost-cell BCs, on z (32,16,8192) fp32.

Strategy (data-parallel over batch, 4 per core x 8 cores; each core splits
its 8192-col domain into 2 halves across partitions -> 128 partitions =
(half, batch, channel), free dim = columns):

The RK3 step is linear in z, so one step collapses to a single 9-tap
convolution:  z' = z + K (*) z,  K = h*C + h^2/2 C^2 + h^3/6 C^3 (radius 4).
K is applied as 9 accumulating matmuls (block-diagonal 128x128 weights, 8
groups of 16 channels) per 512-col PSUM tile + one vector op per tile.
Matmuls run in bfloat16; the systematic weight-rounding error is removed by
one final application of the residual kernel n_steps*(K - bf16(K)).
Boundary-condition-affected cols [10,14) and [L-14,L-10) are recomputed
exactly each step with a stepwise RK3 on a 24-col gathered strip (left edge
in partitions 0..63, right edge in 64..127). Ghost cols 8,9 / L-10,L-9 are
maintained on device (the only ghost cells the interior ever reads); the
full 10-col ghost regions are filled host-side at the end.
"""
import numpy as np
import ml_dtypes

import concourse.bacc as bacc
import concourse.mybir as mybir
from concourse.tile import TileContext
from concourse.bass_utils import run_bass_kernel_spmd

F32 = mybir.dt.float32
BF16 = mybir.dt.bfloat16
IGST = 10
RAD = 4
NFT = 2 * RAD + 1
SW = 24
ALU = mybir.AluOpType
EPS = 0.01
NCORES = 8


def _ksp(Ka, Kb):
    out = {}
    for da, Ma in Ka.items():
        for db, Mb in Kb.items():
            out[da + db] = out.get(da + db, 0) + Ma @ Mb
    return out


def _fused_K(W, h):
    C = {k - 2: W[:, :, k].astype(np.float64) for k in range(5)}
    C2 = _ksp(C, C)
    C3 = _ksp(C2, C)
    K = {}
    for d in set(C) | set(C2) | set(C3):
        if abs(d) <= RAD:
            K[d] = h * C.get(d, 0) + h * h / 2 * C2.get(d, 0) + h ** 3 / 6 * C3.get(d, 0)
    return K


def _blockdiag(mats, n_groups=8):
    P = 16 * n_groups
    out = np.zeros((len(mats), P, P), np.float64)
    for i, blk in enumerate(mats):
        for g in range(n_groups):
            out[i, g * 16:(g + 1) * 16, g * 16:(g + 1) * 16] = blk
    return out.reshape(len(mats) * P, P)


def _make_weights(W, h, n_steps):
    K = _fused_K(W, h)
    mats = [K[j - RAD].T for j in range(NFT)]
    for coef in (h, 4.0 * h):
        for k in range(5):
            mats.append(coef * W[:, :, k].astype(np.float64).T)
    wmain = _blockdiag(mats).astype(np.float32).astype(ml_dtypes.bfloat16)
    resid = []
    for j in range(NFT):
        kb = K[j - RAD].astype(np.float32).astype(ml_dtypes.bfloat16).astype(np.float64)
        resid.append((n_steps * (K[j - RAD] - kb)).T)
    wcorr = _blockdiag(resid).astype(np.float32).astype(ml_dtypes.bfloat16)
    return wmain, wcorr


def _pack_z(z):
    B, nch, L = z.shape
    I = L // 2
    C = I + 8
    rows = np.ascontiguousarray(
        z.reshape(B, 16, 2, I).transpose(2, 0, 1, 3).reshape(128, I), dtype=np.float32)
    buf = np.empty((128, C), np.float32)
    buf[:, 4:4 + I] = rows
    buf[0:64, 0:4] = rows[0:64, 0:4]
    buf[64:128, 0:4] = rows[0:64, I - 4:I]
    buf[0:64, 4 + I:8 + I] = rows[64:128, 0:4]
    buf[64:128, 4 + I:8 + I] = rows[64:128, I - 4:I]
    return buf


def _unpack_z(buf, B, L):
    I = L // 2
    z = buf.reshape(2, B, 16, I).transpose(1, 2, 0, 3).reshape(B, 16, L).copy()
    z[:, :, :IGST] = z[:, :, IGST:IGST + 1]
    z[:, :, L - IGST:] = z[:, :, L - IGST - 1:L - IGST]
    return z


def _build_nc(n_steps, I, TW=512):
    assert I % TW == 0
    NT = I // TW
    C = I + 8
    mmdt = BF16
    nc = bacc.Bacc(None, target_bir_lowering=False)

    zin = nc.declare_dram_parameter("zin", [128, C], F32, isOutput=False)
    win = nc.declare_dram_parameter("wbd", [(NFT + 10) * 128, 128], mmdt, isOutput=False)
    wcin = nc.declare_dram_parameter("wcorr", [NFT * 128, 128], mmdt, isOutput=False)
    zout_d = nc.declare_dram_parameter("zout", [128, I], F32, isOutput=True)

    with TileContext(nc) as tc:
        with tc.tile_pool(name="state", bufs=1) as st, \
             tc.tile_pool(name="wpool", bufs=1) as wp, \
             tc.tile_pool(name="psA", bufs=6, space="PSUM") as psA, \
             tc.tile_pool(name="psS", bufs=2, space="PSUM") as psS:
            za = st.tile([128, C], F32, tag="zA", name="za")
            zb = st.tile([128, C], F32, tag="zB", name="zb")
            zbufs = [za, zb]
            zr = st.tile([128, C], mmdt, tag="zr", name="zr")
            sc32 = st.tile([128, SW], F32, tag="sc32", name="sc32")
            scr = st.tile([128, SW], mmdt, tag="scr", name="scr")
            k1s = st.tile([128, SW], mmdt, tag="k1s", name="k1s")
            k2s = st.tile([128, SW], mmdt, tag="k2s", name="k2s")
            wf, wm, wc = [], {}, []
            for j in range(NFT):
                w = wp.tile([128, 128], mmdt, tag=f"wf{j}", name=f"wf{j}")
                nc.sync.dma_start(out=w[:, :], in_=win[j * 128:(j + 1) * 128, :])
                wf.append(w)
            for v in range(2):
                for k in range(5):
                    idx = NFT + v * 5 + k
                    w = wp.tile([128, 128], mmdt, tag=f"wm{idx}", name=f"wm{idx}")
                    nc.sync.dma_start(out=w[:, :], in_=win[idx * 128:(idx + 1) * 128, :])
                    wm[(v, k)] = w
            for j in range(NFT):
                w = wp.tile([128, 128], mmdt, tag=f"wc{j}", name=f"wc{j}")
                nc.sync.dma_start(out=w[:, :], in_=wcin[j * 128:(j + 1) * 128, :])
                wc.append(w)

            nc.vector.memset(k1s[:, :], 0.0)
            nc.vector.memset(k2s[:, :], 0.0)
            nc.sync.dma_start(out=za[:, :], in_=zin[:, :])
            for t in range(NT):
                s = 4 + TW * t
                nc.scalar.copy(zr[:, s:s + TW], za[:, s:s + TW])
            nc.scalar.copy(zr[:, 0:4], za[:, 0:4])
            nc.scalar.copy(zr[:, 4 + I:C], za[:, 4 + I:C])

            tile_order = list(range(1, NT - 1)) + ([0, NT - 1] if NT > 1 else [0])

            for step in range(n_steps):
                z = zbufs[step % 2]
                znew = zbufs[(step + 1) % 2]

                nc.scalar.copy(sc32[0:64, :], z[0:64, 12:12 + SW])
                nc.scalar.copy(sc32[64:128, :], z[64:128, I - 28:I - 28 + SW])
                nc.scalar.copy(scr[0:64, :], z[0:64, 12:12 + SW])
                nc.scalar.copy(scr[64:128, :], z[64:128, I - 28:I - 28 + SW])
                As = psS.tile([128, SW - 4], F32, tag="As", name=f"As{step}")

                def mini_stage(v, rhs, first=False):
                    for k in range(5):
                        nc.tensor.matmul(As[:, :], wm[(v, k)][:, :],
                                         rhs[:, k:k + SW - 4],
                                         start=(first and k == 0),
                                         stop=(v == 1 and k == 4),
                                         skip_group_check=True)

                def mini_fix(buf):
                    nc.scalar.copy(buf[0:64, 0:2], buf[0:64, 2:3].broadcast_to([64, 2]))
                    nc.scalar.copy(buf[64:128, 22:24],
                                   buf[64:128, 21:22].broadcast_to([64, 2]))

                mini_stage(0, scr, first=True)
                nc.vector.tensor_tensor(k1s[:, 2:22], As[:, :], sc32[:, 2:22], ALU.add)
                mini_fix(k1s)
                mini_stage(0, k1s)
                nc.vector.scalar_tensor_tensor(k2s[:, 2:22], As[:, :], 0.25,
                                               sc32[:, 2:22], ALU.mult, ALU.add)
                mini_fix(k2s)
                mini_stage(1, k2s)

                for t in tile_order:
                    a = psA.tile([128, TW], F32, tag="A", name=f"A{step}_{t}")
                    for j in range(NFT):
                        nc.tensor.matmul(a[:, :], wf[j][:, :],
                                         zr[:, TW * t + j: TW * t + j + TW],
                                         start=(j == 0), stop=(j == NFT - 1),
                                         skip_group_check=True)
                    s = 4 + TW * t
                    nc.vector.tensor_tensor(znew[:, s:s + TW], a[:, :], z[:, s:s + TW],
                                            ALU.add)

                nc.vector.scalar_tensor_tensor(znew[0:64, 14:18], As[0:64, 0:4],
                                               1.0 / 6.0, sc32[0:64, 2:6],
                                               ALU.mult, ALU.add)
                nc.vector.scalar_tensor_tensor(znew[64:128, I - 10:I - 6],
                                               As[64:128, 16:20], 1.0 / 6.0,
                                               sc32[64:128, 18:22],
                                               ALU.mult, ALU.add)

                nc.scalar.copy(znew[0:64, 12:14],
                               znew[0:64, 14:15].broadcast_to([64, 2]))
                nc.scalar.copy(znew[64:128, I - 6:I - 4],
                               znew[64:128, I - 7:I - 6].broadcast_to([64, 2]))
                nc.sync.dma_start(out=znew[0:64, 4 + I:8 + I], in_=znew[64:128, 4:8])
                nc.sync.dma_start(out=znew[64:128, 0:4], in_=znew[0:64, I:I + 4])
                for t in range(NT):
                    s = 4 + TW * t
                    nc.scalar.copy(zr[:, s:s + TW], znew[:, s:s + TW])
                nc.scalar.copy(zr[:, 0:4], znew[:, 0:4])
                nc.scalar.copy(zr[:, 4 + I:C], znew[:, 4 + I:C])

            zfin = zbufs[n_steps % 2]
            zc_out = zbufs[(n_steps + 1) % 2]
            for t in range(NT):
                a = psA.tile([128, TW], F32, tag="A", name=f"Acorr_{t}")
                for j in range(NFT):
                    nc.tensor.matmul(a[:, :], wc[j][:, :],
                                     zr[:, TW * t + j: TW * t + j + TW],
                                     start=(j == 0), stop=(j == NFT - 1),
                                     skip_group_check=True)
                s = 4 + TW * t
                nc.vector.tensor_tensor(zc_out[:, s:s + TW], a[:, :],
                                        zfin[:, s:s + TW], ALU.add)
            nc.sync.dma_start(out=zout_d[:, :], in_=zc_out[:, 4:4 + I])

    nc.finalize()
    return nc


_NC_CACHE = {}


def kernel(z0, W, t1_t0):
    z0 = np.asarray(z0, dtype=np.float32)
    W = np.asarray(W, dtype=np.float32)
    t = int(np.asarray(t1_t0))
    if t == 0:
        return z0.copy()
    n_steps = int(round(t / EPS))
    h = t / n_steps
    B, nch, L = z0.shape
    I = L // 2
    BPC = B // NCORES

    key = (n_steps, I)
    if key not in _NC_CACHE:
        _NC_CACHE[key] = _build_nc(n_steps, I)
    nc = _NC_CACHE[key]

    wmain, wcorr = _make_weights(W, h, n_steps)
    in_maps = [{"zin": _pack_z(z0[c * BPC:(c + 1) * BPC]),
                "wbd": wmain, "wcorr": wcorr} for c in range(NCORES)]
    br = run_bass_kernel_spmd(nc, in_maps, list(range(NCORES)))
    out = np.concatenate(
        [_unpack_z(br.results[c]["zout"], BPC, L) for c in range(NCORES)], axis=0)
    return out.astype(np.float32)


# revision 1
# speedup vs baseline: 1.0052x; 1.0052x over previous
"""nn_NeuralODE TRN2 kernel: 100 SSP-RK3 steps of a learned 16-channel
1D stencil (conv k=5) with ghost-cell BCs, on z (32,16,8192) fp32.

Strategy (data-parallel over batch, 4 per core x 8 cores; each core splits
its 8192-col domain into 2 halves across partitions -> 128 partitions =
(half, batch, channel), free dim = columns):

The RK3 step is linear in z, so one step collapses to a single 9-tap
convolution:  z' = z + K (*) z,  K = h*C + h^2/2 C^2 + h^3/6 C^3 (radius 4).
K is applied as 9 accumulating matmuls (block-diagonal 128x128 weights, 8
groups of 16 channels) per 512-col PSUM tile + one vector op per tile.
Matmuls run in bfloat16; the systematic weight-rounding error is removed by
one final application of the residual kernel n_steps*(K - bf16(K)).
Boundary-condition-affected cols [10,14) and [L-14,L-10) are recomputed
exactly each step with a stepwise RK3 on a 24-col gathered strip (left edge
in partitions 0..63, right edge in 64..127). Ghost cols 8,9 / L-10,L-9 are
maintained on device (the only ghost cells the interior ever reads); the
full 10-col ghost regions are filled host-side at the end.
"""
import numpy as np
import ml_dtypes

import concourse.bacc as bacc
import concourse.mybir as mybir
from concourse.tile import TileContext
from concourse.bass_utils import run_bass_kernel_spmd

F32 = mybir.dt.float32
BF16 = mybir.dt.bfloat16
IGST = 10
RAD = 4
NFT = 2 * RAD + 1
SW = 24
ALU = mybir.AluOpType
EPS = 0.01
NCORES = 8


def _ksp(Ka, Kb):
    out = {}
    for da, Ma in Ka.items():
        for db, Mb in Kb.items():
            out[da + db] = out.get(da + db, 0) + Ma @ Mb
    return out


def _fused_K(W, h):
    C = {k - 2: W[:, :, k].astype(np.float64) for k in range(5)}
    C2 = _ksp(C, C)
    C3 = _ksp(C2, C)
    K = {}
    for d in set(C) | set(C2) | set(C3):
        if abs(d) <= RAD:
            K[d] = h * C.get(d, 0) + h * h / 2 * C2.get(d, 0) + h ** 3 / 6 * C3.get(d, 0)
    return K


def _blockdiag(mats, n_groups=8):
    P = 16 * n_groups
    out = np.zeros((len(mats), P, P), np.float64)
    for i, blk in enumerate(mats):
        for g in range(n_groups):
            out[i, g * 16:(g + 1) * 16, g * 16:(g + 1) * 16] = blk
    return out.reshape(len(mats) * P, P)


def _make_weights(W, h, n_steps):
    K = _fused_K(W, h)
    mats = [K[j - RAD].T for j in range(NFT)]
    for coef in (h, 4.0 * h):
        for k in range(5):
            mats.append(coef * W[:, :, k].astype(np.float64).T)
    wmain = _blockdiag(mats).astype(np.float32).astype(ml_dtypes.bfloat16)
    resid = []
    for j in range(NFT):
        kb = K[j - RAD].astype(np.float32).astype(ml_dtypes.bfloat16).astype(np.float64)
        resid.append((n_steps * (K[j - RAD] - kb)).T)
    wcorr = _blockdiag(resid).astype(np.float32).astype(ml_dtypes.bfloat16)
    return wmain, wcorr


def _pack_z(z):
    B, nch, L = z.shape
    I = L // 2
    C = I + 8
    rows = np.ascontiguousarray(
        z.reshape(B, 16, 2, I).transpose(2, 0, 1, 3).reshape(128, I), dtype=np.float32)
    buf = np.empty((128, C), np.float32)
    buf[:, 4:4 + I] = rows
    buf[0:64, 0:4] = rows[0:64, 0:4]
    buf[64:128, 0:4] = rows[0:64, I - 4:I]
    buf[0:64, 4 + I:8 + I] = rows[64:128, 0:4]
    buf[64:128, 4 + I:8 + I] = rows[64:128, I - 4:I]
    return buf


def _unpack_z(buf, B, L):
    I = L // 2
    z = buf.reshape(2, B, 16, I).transpose(1, 2, 0, 3).reshape(B, 16, L).copy()
    z[:, :, :IGST] = z[:, :, IGST:IGST + 1]
    z[:, :, L - IGST:] = z[:, :, L - IGST - 1:L - IGST]
    return z


def _build_nc(n_steps, I, TW=512):
    assert I % TW == 0
    NT = I // TW
    C = I + 8
    mmdt = BF16
    nc = bacc.Bacc(None, target_bir_lowering=False)

    zin = nc.declare_dram_parameter("zin", [128, C], F32, isOutput=False)
    win = nc.declare_dram_parameter("wbd", [(NFT + 10) * 128, 128], mmdt, isOutput=False)
    wcin = nc.declare_dram_parameter("wcorr", [NFT * 128, 128], mmdt, isOutput=False)
    zout_d = nc.declare_dram_parameter("zout", [128, I], F32, isOutput=True)

    with TileContext(nc) as tc:
        with tc.tile_pool(name="state", bufs=1) as st, \
             tc.tile_pool(name="wpool", bufs=1) as wp, \
             tc.tile_pool(name="psA", bufs=6, space="PSUM") as psA, \
             tc.tile_pool(name="psS", bufs=2, space="PSUM") as psS:
            za = st.tile([128, C], F32, tag="zA", name="za")
            zb = st.tile([128, C], F32, tag="zB", name="zb")
            zbufs = [za, zb]
            zr = st.tile([128, C], mmdt, tag="zr", name="zr")
            sc32 = st.tile([128, SW], F32, tag="sc32", name="sc32")
            scr = st.tile([128, SW], mmdt, tag="scr", name="scr")
            k1s = st.tile([128, SW], mmdt, tag="k1s", name="k1s")
            k2s = st.tile([128, SW], mmdt, tag="k2s", name="k2s")
            wf, wm, wc = [], {}, []
            for j in range(NFT):
                w = wp.tile([128, 128], mmdt, tag=f"wf{j}", name=f"wf{j}")
                nc.sync.dma_start(out=w[:, :], in_=win[j * 128:(j + 1) * 128, :])
                wf.append(w)
            for v in range(2):
                for k in range(5):
                    idx = NFT + v * 5 + k
                    w = wp.tile([128, 128], mmdt, tag=f"wm{idx}", name=f"wm{idx}")
                    nc.sync.dma_start(out=w[:, :], in_=win[idx * 128:(idx + 1) * 128, :])
                    wm[(v, k)] = w
            for j in range(NFT):
                w = wp.tile([128, 128], mmdt, tag=f"wc{j}", name=f"wc{j}")
                nc.sync.dma_start(out=w[:, :], in_=wcin[j * 128:(j + 1) * 128, :])
                wc.append(w)

            nc.vector.memset(k1s[:, :], 0.0)
            nc.vector.memset(k2s[:, :], 0.0)
            nc.sync.dma_start(out=za[:, :], in_=zin[:, :])
            for t in range(NT):
                s = 4 + TW * t
                nc.scalar.copy(zr[:, s:s + TW], za[:, s:s + TW])
            nc.scalar.copy(zr[:, 0:4], za[:, 0:4])
            nc.scalar.copy(zr[:, 4 + I:C], za[:, 4 + I:C])

            tile_order = list(range(1, NT - 1)) + ([0, NT - 1] if NT > 1 else [0])

            for step in range(n_steps):
                z = zbufs[step % 2]
                znew = zbufs[(step + 1) % 2]

                nc.scalar.copy(sc32[0:64, :], z[0:64, 12:12 + SW])
                nc.scalar.copy(sc32[64:128, :], z[64:128, I - 28:I - 28 + SW])
                nc.scalar.copy(scr[0:64, :], z[0:64, 12:12 + SW])
                nc.scalar.copy(scr[64:128, :], z[64:128, I - 28:I - 28 + SW])
                As = psS.tile([128, SW - 4], F32, tag="As", name=f"As{step}")

                def mini_stage(v, rhs, first=False):
                    for k in range(5):
                        nc.tensor.matmul(As[:, :], wm[(v, k)][:, :],
                                         rhs[:, k:k + SW - 4],
                                         start=(first and k == 0),
                                         stop=(v == 1 and k == 4),
                                         skip_group_check=True)

                def mini_fix(buf):
                    nc.scalar.copy(buf[0:64, 0:2], buf[0:64, 2:3].broadcast_to([64, 2]))
                    nc.scalar.copy(buf[64:128, 22:24],
                                   buf[64:128, 21:22].broadcast_to([64, 2]))

                mini_stage(0, scr, first=True)
                nc.vector.tensor_tensor(k1s[:, 2:22], As[:, :], sc32[:, 2:22], ALU.add)
                mini_fix(k1s)
                mini_stage(0, k1s)
                nc.vector.scalar_tensor_tensor(k2s[:, 2:22], As[:, :], 0.25,
                                               sc32[:, 2:22], ALU.mult, ALU.add)
                mini_fix(k2s)
                mini_stage(1, k2s)

                for t in tile_order:
                    a = psA.tile([128, TW], F32, tag="A", name=f"A{step}_{t}")
                    for j in range(NFT):
                        nc.tensor.matmul(a[:, :], wf[j][:, :],
                                         zr[:, TW * t + j: TW * t + j + TW],
                                         start=(j == 0), stop=(j == NFT - 1),
                                         skip_group_check=True)
                    s = 4 + TW * t
                    nc.vector.tensor_tensor(znew[:, s:s + TW], a[:, :], z[:, s:s + TW],
                                            ALU.add)

                nc.vector.scalar_tensor_tensor(znew[0:64, 14:18], As[0:64, 0:4],
                                               1.0 / 6.0, sc32[0:64, 2:6],
                                               ALU.mult, ALU.add)
                nc.vector.scalar_tensor_tensor(znew[64:128, I - 10:I - 6],
                                               As[64:128, 16:20], 1.0 / 6.0,
                                               sc32[64:128, 18:22],
                                               ALU.mult, ALU.add)

                nc.scalar.copy(znew[0:64, 12:14],
                               znew[0:64, 14:15].broadcast_to([64, 2]))
                nc.scalar.copy(znew[64:128, I - 6:I - 4],
                               znew[64:128, I - 7:I - 6].broadcast_to([64, 2]))
                nc.sync.dma_start(out=znew[0:64, 4 + I:8 + I], in_=znew[64:128, 4:8])
                nc.sync.dma_start(out=znew[64:128, 0:4], in_=znew[0:64, I:I + 4])
                for t in range(NT):
                    s = 4 + TW * t
                    nc.scalar.copy(zr[:, s:s + TW], znew[:, s:s + TW])
                nc.scalar.copy(zr[:, 0:4], znew[:, 0:4])
                nc.scalar.copy(zr[:, 4 + I:C], znew[:, 4 + I:C])

            zfin = zbufs[n_steps % 2]
            zc_out = zbufs[(n_steps + 1) % 2]
            for t in range(NT):
                a = psA.tile([128, TW], F32, tag="A", name=f"Acorr_{t}")
                for j in range(NFT):
                    nc.tensor.matmul(a[:, :], wc[j][:, :],
                                     zr[:, TW * t + j: TW * t + j + TW],
                                     start=(j == 0), stop=(j == NFT - 1),
                                     skip_group_check=True)
                s = 4 + TW * t
                nc.vector.tensor_tensor(zc_out[:, s:s + TW], a[:, :],
                                        zfin[:, s:s + TW], ALU.add)
            nc.sync.dma_start(out=zout_d[:, :], in_=zc_out[:, 4:4 + I])

    nc.finalize()
    return nc


_NC_CACHE = {}


def kernel(z0, W, t1_t0):
    z0 = np.asarray(z0, dtype=np.float32)
    W = np.asarray(W, dtype=np.float32)
    t = int(np.asarray(t1_t0))
    if t == 0:
        return z0.copy()
    n_steps = int(round(t / EPS))
    h = t / n_steps
    B, nch, L = z0.shape
    I = L // 2
    BPC = B // NCORES

    key = (n_steps, I)
    if key not in _NC_CACHE:
        _NC_CACHE[key] = _build_nc(n_steps, I)
    nc = _NC_CACHE[key]

    wmain, wcorr = _make_weights(W, h, n_steps)
    in_maps = [{"zin": _pack_z(z0[c * BPC:(c + 1) * BPC]),
                "wbd": wmain, "wcorr": wcorr} for c in range(NCORES)]
    br = run_bass_kernel_spmd(nc, in_maps, list(range(NCORES)))
    out = np.concatenate(
        [_unpack_z(br.results[c]["zout"], BPC, L) for c in range(NCORES)], axis=0)
    return out.astype(np.float32)
